# revision 1
# baseline (speedup 1.0000x reference)
"""CrossAttention kernel for 8 trn2 NeuronCores.

Sharding: core c handles batch b = c//4 and head group hg = c%4 (4 of 16 heads).
Within a 4-core group (one batch), the output projection partials are
ReduceScattered over the sequence dim; each core applies the final LayerNorm to
its 512-row slice and returns it.  The host reassembles the full output.

All heavy matmuls run in float32r (single-pass reduced-precision fp32,
~1.6e-4 rel err, 4x faster than fp32 on the PE).
"""

import sys

sys.path.insert(0, "/opt/trn_rl_repo")

import numpy as np

import concourse.bass as bass
import concourse.mybir as mybir
import concourse.tile as tile
from concourse.bass_utils import run_bass_kernel_spmd
from concourse.masks import make_identity

# problem constants (hardcoded per the harness contract)
B, N, M, DIM = 2, 2048, 2048, 1024
HEADS, DH = 16, 64
INNER = HEADS * DH
H_PER = HEADS // 8 * 2  # 4 heads per core (16 heads / 8 cores * 2 batches)
HS = H_PER * DH  # 256 inner columns per core
NT = N // 128  # 16 seq tiles
KT = DIM // 128  # 8 contraction tiles
QC = N // 512  # 4 query chunks
EPS = 1e-5
SCALE = DH ** -0.5
NEG_BIG = -1.0e30

F32 = mybir.dt.float32
F32R = mybir.dt.float32r

_cache = {}


def split_multi_waits(nc):
    """This container's walrus supports a single sync-wait per instruction.
    Move extra waits onto same-engine NOPs placed immediately before."""
    for f in nc.m.functions:
        for blk in f.blocks:
            insts = list(blk.instructions)
            if not any(
                i.sync_info is not None and len(i.sync_info.on_wait) > 1
                for i in insts
            ):
                continue
            new_list = []
            for inst in insts:
                si = inst.sync_info
                if si is not None and len(si.on_wait) > 1:
                    waits = list(si.on_wait)
                    for k, w in enumerate(waits[:-1]):
                        new_list.append(
                            mybir.InstNoOp(
                                name=f"{inst.name}_ws{k}",
                                sync_info=mybir.SyncInfo(on_wait=[w], on_update=[]),
                                bass_nofuse=True,
                                engine=inst.engine,
                            )
                        )
                    inst.sync_info = mybir.SyncInfo(
                        on_wait=[waits[-1]], on_update=list(si.on_update)
                    )
                new_list.append(inst)
            blk.instructions = new_list


def build_program():
    nc = bass.Bass("TRN2", target_bir_lowering=False, debug=False, num_devices=8)
    AF = mybir.ActivationFunctionType

    x = nc.dram_tensor("x", [N, DIM], F32, kind="ExternalInput")
    ctx_in = nc.dram_tensor("ctx", [M, DIM], F32, kind="ExternalInput")
    maskbias = nc.dram_tensor("maskbias", [128, NT + 1], F32, kind="ExternalInput")
    nk_in = nc.dram_tensor("nk", [DH, 1], F32, kind="ExternalInput")
    nvr_in = nc.dram_tensor("nvr", [1, DH + 1], F32, kind="ExternalInput")
    ones64_in = nc.dram_tensor("ones64", [1, DH], F32, kind="ExternalInput")
    wq_in = nc.dram_tensor("wq", [DIM, HS], F32, kind="ExternalInput")
    wk_in = nc.dram_tensor("wk", [DIM, HS], F32, kind="ExternalInput")
    wv_in = nc.dram_tensor("wv", [DIM, HS], F32, kind="ExternalInput")
    wout_in = nc.dram_tensor("wout", [HS, DIM], F32, kind="ExternalInput")
    gout_in = nc.dram_tensor("gout", [DIM], F32, kind="ExternalInput")
    y = nc.dram_tensor("y", [N // 4, DIM], F32, kind="ExternalOutput")

    with tile.TileContext(nc) as tc:
        with tc.tile_pool(name="persist", bufs=1) as persist, \
             tc.tile_pool(name="dram", bufs=1, space="DRAM") as dram:
            ident = persist.tile([128, 128], F32)
            make_identity(nc, ident[:])
            eps_t = persist.tile([128, 1], F32)
            nc.vector.memset(eps_t[:], EPS)

            # per-head transposed projections (partitions = head dim 0..63)
            qT = persist.tile([DH, H_PER, N], F32R)
            kT = persist.tile([DH, H_PER, M + 1], F32R)  # col M = null key
            vhat = persist.tile([128, H_PER, NT, DH + 1], F32R)  # ones col at DH
            mb = persist.tile([128, NT + 1], F32)
            nc.scalar.dma_start(mb[:], maskbias[:])
            nvr = persist.tile([1, DH + 1], F32R)
            nc.scalar.dma_start(nvr[:], nvr_in[:].bitcast(F32R))
            ones64 = persist.tile([1, DH], F32R)
            nc.scalar.dma_start(ones64[:], ones64_in[:].bitcast(F32R))
            nc.sync.dma_start(
                kT[:, :, M : M + 1],
                nk_in[:].bitcast(F32R).unsqueeze(1).broadcast_to([DH, H_PER, 1]),
            )
            # ones column of vhat (before v blocks overwrite cols 0..DH-1)
            ones_f = persist.tile([128, 1], F32)
            nc.vector.memset(ones_f[:], 1.0)
            for h in range(H_PER):
                for t in range(NT):
                    nc.vector.tensor_copy(vhat[:, h, t, DH : DH + 1], ones_f[:])

            # ---------------- Phase A: x -> LN -> transpose -> qT -----------
            with tc.tile_pool(name="pha", bufs=3) as pha, \
                 tc.tile_pool(name="pha1", bufs=3) as pha1, \
                 tc.tile_pool(name="phas", bufs=4) as phas, \
                 tc.tile_pool(name="xnT_p", bufs=1) as xnT_p, \
                 tc.tile_pool(name="wq_p", bufs=1) as wq_p, \
                 tc.tile_pool(name="ps_tp", bufs=4, space="PSUM") as ps_tp, \
                 tc.tile_pool(name="ps_pr", bufs=2, space="PSUM") as ps_pr:
                xnT = xnT_p.tile([128, KT, N], F32R)
                wq = wq_p.tile([128, KT, HS], F32R)
                nc.scalar.dma_start(
                    wq[:], wq_in[:].bitcast(F32R).rearrange("(t p) n -> p t n", p=128)
                )
                for t in range(NT):
                    x_t = pha.tile([128, DIM], F32, tag="x_t")
                    nc.sync.dma_start(x_t[:], x[t * 128 : (t + 1) * 128, :])
                    stats = phas.tile([128, 2, 6], F32, tag="stats")
                    xr = x_t[:].rearrange("p (s d) -> p s d", d=512)
                    for s in range(2):
                        nc.vector.bn_stats(stats[:, s, :], xr[:, s, :])
                    mv = phas.tile([128, 2], F32, tag="mv")
                    nc.vector.bn_aggr(mv[:], stats[:])
                    # rstd*scale = exp(-0.5*ln(var+eps)) * SCALE
                    lnv = phas.tile([128, 1], F32, tag="lnv")
                    nc.scalar.activation(lnv[:], mv[:, 1:2], AF.Ln, bias=eps_t[:])
                    c_t = phas.tile([128, 1], F32, tag="c_t")
                    nc.scalar.activation(c_t[:], lnv[:], AF.Exp, scale=-0.5)
                    cs_t = phas.tile([128, 1], F32, tag="cs_t")
                    nc.scalar.mul(cs_t[:], c_t[:], SCALE)
                    nmc = phas.tile([128, 1], F32, tag="nmc")
                    nc.vector.scalar_tensor_tensor(
                        out=nmc[:], in0=mv[:, 0:1], scalar=-1.0, in1=cs_t[:],
                        op0=mybir.AluOpType.mult, op1=mybir.AluOpType.mult,
                    )
                    xs_t = pha1.tile([128, DIM], F32, tag="xs_t")
                    nc.scalar.activation(
                        xs_t[:], x_t[:], AF.Identity, bias=nmc[:], scale=cs_t[:]
                    )
                    for d in range(KT):
                        pst = ps_tp.tile([128, 128], F32, tag="tp")
                        nc.tensor.transpose(
                            pst[:], xs_t[:, d * 128 : (d + 1) * 128], ident[:]
                        )
                        nc.vector.tensor_copy(
                            xnT[:, d, t * 128 : (t + 1) * 128], pst[:]
                        )
                # q projection: head pairs packed on psum partitions
                for p in range(H_PER // 2):
                    for qc in range(QC):
                        psq = ps_pr.tile([128, 512], F32, tag="psq")
                        for k in range(KT):
                            nc.tensor.matmul(
                                psq[:],
                                wq[:, k, p * 128 : (p + 1) * 128],
                                xnT[:, k, qc * 512 : (qc + 1) * 512],
                                start=(k == 0), stop=(k == KT - 1),
                            )
                        nc.vector.tensor_copy(
                            qT[:, 2 * p, qc * 512 : (qc + 1) * 512], psq[0:DH, :]
                        )
                        nc.vector.tensor_copy(
                            qT[:, 2 * p + 1, qc * 512 : (qc + 1) * 512], psq[DH:128, :]
                        )

            # ---------------- Phase B: ctx -> transpose -> kT, vhat ---------
            with tc.tile_pool(name="phb", bufs=3) as phb, \
                 tc.tile_pool(name="ctxT_p", bufs=1) as ctxT_p, \
                 tc.tile_pool(name="wkv_p", bufs=1) as wkv_p, \
                 tc.tile_pool(name="ps_tp2", bufs=4, space="PSUM") as ps_tp2, \
                 tc.tile_pool(name="ps_pr2", bufs=2, space="PSUM") as ps_pr2, \
                 tc.tile_pool(name="ps_v", bufs=2, space="PSUM") as ps_v:
                ctxT = ctxT_p.tile([128, KT, M], F32R)
                wk = wkv_p.tile([128, KT, HS], F32R, tag="wk")
                wv = wkv_p.tile([128, KT, HS], F32R, tag="wv")
                nc.scalar.dma_start(
                    wk[:], wk_in[:].bitcast(F32R).rearrange("(t p) n -> p t n", p=128)
                )
                nc.scalar.dma_start(
                    wv[:], wv_in[:].bitcast(F32R).rearrange("(t p) n -> p t n", p=128)
                )
                for t in range(NT):
                    c_t = phb.tile([128, DIM], F32, tag="c_t")
                    nc.scalar.dma_start(c_t[:], ctx_in[t * 128 : (t + 1) * 128, :])
                    for d in range(KT):
                        pst = ps_tp2.tile([128, 128], F32, tag="tp2")
                        nc.tensor.transpose(
                            pst[:], c_t[:, d * 128 : (d + 1) * 128], ident[:]
                        )
                        nc.vector.tensor_copy(
                            ctxT[:, d, t * 128 : (t + 1) * 128], pst[:]
                        )
                # k projection (head pairs)
                for p in range(H_PER // 2):
                    for qc in range(QC):
                        psk = ps_pr2.tile([128, 512], F32, tag="psk")
                        for k in range(KT):
                            nc.tensor.matmul(
                                psk[:],
                                wk[:, k, p * 128 : (p + 1) * 128],
                                ctxT[:, k, qc * 512 : (qc + 1) * 512],
                                start=(k == 0), stop=(k == KT - 1),
                            )
                        nc.vector.tensor_copy(
                            kT[:, 2 * p, qc * 512 : (qc + 1) * 512], psk[0:DH, :]
                        )
                        nc.vector.tensor_copy(
                            kT[:, 2 * p + 1, qc * 512 : (qc + 1) * 512], psk[DH:128, :]
                        )
                # v natural: [keys, dh] per head, ctxT as stationary
                for t in range(NT):
                    psv = ps_v.tile([128, HS], F32, tag="psv")
                    for k in range(KT):
                        nc.tensor.matmul(
                            psv[:],
                            ctxT[:, k, t * 128 : (t + 1) * 128],
                            wv[:, k, :],
                            start=(k == 0), stop=(k == KT - 1),
                        )
                    for h in range(H_PER):
                        nc.vector.tensor_copy(
                            vhat[:, h, t, 0:DH], psv[:, h * DH : (h + 1) * DH]
                        )

            # ---------------- Phase C: attention per head -------------------
            # outT lives across phases C and D only (saves 32KB during A/B)
            outT_cm = tc.tile_pool(name="outT_p", bufs=1)
            outT_pool = outT_cm.__enter__()
            outT = outT_pool.tile([DH, H_PER, N], F32R)
            with tc.tile_pool(name="phc", bufs=3) as phc, \
                 tc.tile_pool(name="phc2", bufs=2) as phc2, \
                 tc.tile_pool(name="ps_sim", bufs=2, space="PSUM") as ps_sim, \
                 tc.tile_pool(name="ps_out", bufs=1, space="PSUM") as ps_out:
                for h in range(H_PER):
                    pso = ps_out.tile([DH + 1, N], F32, tag="pso")
                    for t in range(NT + 1):
                        if t < NT:
                            rows = 128
                            lhs = kT[:, h, t * 128 : (t + 1) * 128]
                            vrow = vhat[:, h, t, :]
                        else:
                            rows = 1
                            lhs = kT[:, h, M : M + 1]
                            vrow = nvr[:]
                        # 1024-wide halves, double-buffered so the next
                        # sim matmuls overlap this half's exp
                        for hf in range(2):
                            pss = ps_sim.tile([rows, N // 2], F32, tag="sim")
                            for qc in range(2):
                                q0 = hf * 1024 + qc * 512
                                nc.tensor.matmul(
                                    pss[:, qc * 512 : (qc + 1) * 512],
                                    lhs,
                                    qT[:, h, q0 : q0 + 512],
                                    start=True, stop=True,
                                )
                            pt = phc.tile([rows, N // 2], F32R, tag="pt")
                            nc.scalar.activation(
                                pt[:], pss[:], AF.Exp,
                                bias=mb[0:rows, t : t + 1],
                            )
                            for qc in range(2):
                                q0 = hf * 1024 + qc * 512
                                nc.tensor.matmul(
                                    pso[:, q0 : q0 + 512],
                                    vrow,
                                    pt[:, qc * 512 : (qc + 1) * 512],
                                    start=(t == 0), stop=(t == NT),
                                )
                    # divide by row sums (pso row DH) and store out_hT
                    rec = phc2.tile([1, N], F32R, tag="rec")
                    with nc.allow_low_precision(reason="f32r rounding"):
                        nc.vector.reciprocal(rec[:], pso[DH : DH + 1, :])
                    psb0 = ps_sim.tile([DH, N // 2], F32, tag="sim")
                    psb1 = ps_sim.tile([DH, N // 2], F32, tag="sim")
                    for qc, psb_h in [(0, psb0), (1, psb0), (2, psb1), (3, psb1)]:
                        nc.tensor.matmul(
                            psb_h[:, (qc % 2) * 512 : (qc % 2 + 1) * 512],
                            ones64[:],
                            rec[:, qc * 512 : (qc + 1) * 512],
                            start=True, stop=True,
                        )
                    o_s = phc2.tile([DH, N], F32, tag="o_s")
                    nc.vector.tensor_copy(o_s[:], pso[0:DH, :])
                    for hf, psb_h in [(0, psb0), (1, psb1)]:
                        nc.vector.tensor_tensor(
                            out=outT[:, h, hf * 1024 : (hf + 1) * 1024],
                            in0=o_s[:, hf * 1024 : (hf + 1) * 1024],
                            in1=psb_h[:],
                            op=mybir.AluOpType.mult,
                        )

            # ---------------- Phase D: out proj + RS + final LN -------------
            partial = dram.tile([N, DIM], F32)
            rs_out = dram.tile([N // 4, DIM], F32)
            with tc.tile_pool(name="phd", bufs=2) as phd, \
                 tc.tile_pool(name="wout_p", bufs=1) as wout_p, \
                 tc.tile_pool(name="ps_d", bufs=4, space="PSUM") as ps_d:
                wout = wout_p.tile([DH, H_PER, DIM], F32R)
                nc.scalar.dma_start(
                    wout[:],
                    wout_in[:].bitcast(F32R).rearrange("(h p) n -> p h n", p=DH),
                )
                for st in range(NT):
                    part_s = phd.tile([128, DIM], F32, tag="part_s")
                    for ch in range(2):
                        psp = ps_d.tile([128, 512], F32, tag="psp")
                        for h in range(H_PER):
                            nc.tensor.matmul(
                                psp[:],
                                outT[:, h, st * 128 : (st + 1) * 128],
                                wout[:, h, ch * 512 : (ch + 1) * 512],
                                start=(h == 0), stop=(h == H_PER - 1),
                            )
                        nc.vector.tensor_copy(
                            part_s[:, ch * 512 : (ch + 1) * 512], psp[:]
                        )
                    nc.gpsimd.dma_start(
                        partial[st * 128 : (st + 1) * 128, :], part_s[:]
                    )
                nc.gpsimd.collective_compute(
                    "ReduceScatter",
                    mybir.AluOpType.add,
                    replica_groups=[[0, 1, 2, 3], [4, 5, 6, 7]],
                    ins=[partial[:].opt()],
                    outs=[rs_out[:].opt()],
                )
                # final LN on rs_out [512, 1024]
                gout_b = wout_p.tile([128, DIM], F32, tag="gout_b")
                nc.sync.dma_start(
                    gout_b[:], gout_in[:].unsqueeze(0).broadcast_to([128, DIM])
                )
                for t in range(N // 4 // 128):
                    y_t = phd.tile([128, DIM], F32, tag="y_t")
                    nc.gpsimd.dma_start(y_t[:], rs_out[t * 128 : (t + 1) * 128, :])
                    stats = phd.tile([128, 2, 6], F32, tag="statsd")
                    yr = y_t[:].rearrange("p (s d) -> p s d", d=512)
                    for s in range(2):
                        nc.vector.bn_stats(stats[:, s, :], yr[:, s, :])
                    mv = phd.tile([128, 2], F32, tag="mvd")
                    nc.vector.bn_aggr(mv[:], stats[:])
                    lnv = phd.tile([128, 1], F32, tag="lnvd")
                    nc.scalar.activation(lnv[:], mv[:, 1:2], AF.Ln, bias=eps_t[:])
                    rstd = phd.tile([128, 1], F32, tag="rstdd")
                    nc.scalar.activation(rstd[:], lnv[:], AF.Exp, scale=-0.5)
                    nc.vector.tensor_scalar(
                        out=y_t[:], in0=y_t[:], scalar1=mv[:, 0:1], scalar2=rstd[:],
                        op0=mybir.AluOpType.subtract, op1=mybir.AluOpType.mult,
                    )
                    yo = phd.tile([128, DIM], F32, tag="yo")
                    nc.vector.tensor_tensor(
                        out=yo[:], in0=y_t[:], in1=gout_b[:],
                        op=mybir.AluOpType.mult,
                    )
                    nc.gpsimd.dma_start(y[t * 128 : (t + 1) * 128, :], yo[:])
            outT_cm.__exit__(None, None, None)

    split_multi_waits(nc)
    return nc


def _prep_inputs(x, context, mask, g_norm, null_kv, Wq, Wkv, Wout, g_out):
    """Host-side sharding: slice weights/activations per core."""
    x = np.asarray(x, dtype=np.float32)
    context = np.asarray(context, dtype=np.float32)
    mask = np.asarray(mask)
    g_norm = np.asarray(g_norm, dtype=np.float32)
    null_kv = np.asarray(null_kv, dtype=np.float32)
    Wq = np.asarray(Wq, dtype=np.float32)
    Wkv = np.asarray(Wkv, dtype=np.float32)
    Wout = np.asarray(Wout, dtype=np.float32)
    g_out = np.asarray(g_out, dtype=np.float32)

    Wq_g = (g_norm[:, None] * Wq).astype(np.float32)  # fold g_norm into Wq
    nk = np.ascontiguousarray(null_kv[0].reshape(DH, 1))
    nvr = np.concatenate([null_kv[1], [1.0]]).reshape(1, DH + 1).astype(np.float32)
    ones64 = np.ones((1, DH), np.float32)

    in_maps = []
    for c in range(8):
        b, hg = c // 4, c % 4
        hs = hg * HS
        bias = np.where(mask[b], 0.0, NEG_BIG).astype(np.float32)  # [M]
        mb = np.zeros((128, NT + 1), np.float32)
        mb[:, :NT] = bias.reshape(NT, 128).T
        in_maps.append(
            {
                "x": np.ascontiguousarray(x[b]),
                "ctx": np.ascontiguousarray(context[b]),
                "maskbias": mb,
                "nk": nk,
                "nvr": nvr,
                "ones64": ones64,
                "wq": np.ascontiguousarray(Wq_g[:, hs : hs + HS]),
                "wk": np.ascontiguousarray(Wkv[:, hs : hs + HS]),
                "wv": np.ascontiguousarray(Wkv[:, INNER + hs : INNER + hs + HS]),
                "wout": np.ascontiguousarray(Wout[hs : hs + HS, :]),
                "gout": g_out,
            }
        )
    return in_maps


def _get_program():
    if "nc" not in _cache:
        _cache["nc"] = build_program()
    return _cache["nc"]


def kernel(x, context, mask, g_norm, null_kv, Wq, Wkv, Wout, g_out, _trace=False):
    nc = _get_program()
    in_maps = _prep_inputs(x, context, mask, g_norm, null_kv, Wq, Wkv, Wout, g_out)
    res = run_bass_kernel_spmd(nc, in_maps, list(range(8)), trace=_trace)
    out = np.empty((B, N, DIM), np.float32)
    for c in range(8):
        b, idx = c // 4, c % 4
        out[b, idx * 512 : (idx + 1) * 512, :] = res.results[c]["y"]
    if _trace:
        return out, res
    return out

